# revision 1
# baseline (speedup 1.0000x reference)
"""DGL capsule routing layer on 8 trn2 NeuronCores (Bass/Tile).

Math: for routing_num iterations,
    c = softmax(b, axis=out)                        # b0 = 0
    s = einsum('io,iof->of', c, uh)
    v = squash(s)
    b = b + einsum('iof,of->io', uh, v)
Output: final v [OUT, F].

Key identity: b_t = uh . (v_1 + ... + v_t)  (b is linear in uh), so b is
never materialized across iterations; each iteration is one streaming pass
over uh with w_t = cumulative sum of v's:
    pass t: b = sum_f uh[i,o,f]*w[o,f]; e = exp(b); r_i = 1/sum_o e
            s[o,f] = sum_i r_i * e[i,o] * uh[i,o,f]   (partial per core)
            AllReduce(s); v = squash(s); w += v
Pass 1 has c uniform (=1/OUT) so it is a pure PE pass.

Sharding: i (in_nodes) split across 8 cores, 512 rows each (4 blocks of
128 partitions). Engine plan per 2048-wide o-f chunk (passes >= 2):
  GpSimd: tm = uh * w_bcast        (2-input mul; DVE TT never contends)
  DVE:    b-slice = segsum_f(tm);  p = e * uh (e broadcast over f)
  ACT:    e = exp(b) with fused denominator accum; psum flushes
  PE:     s-partial = sum_i rinv[i]*p[i,:] as 4x N=512 matmuls with
          rinv as the 1-column stationary operand -> psum [1,2048]
The per-block s partials go straight to DRAM [4,16384]; the AllReduce sums
over cores, and the cheap cross-block sum happens after the AR in the
partition-spread [128,128] layout (3 DVE adds).
"""

import numpy as np
from contextlib import ExitStack

import concourse.bass as bass
import concourse.mybir as mybir
import concourse.tile as tile
from concourse import bacc
from concourse import bass_utils

F32 = mybir.dt.float32
AX = mybir.AxisListType
AF = mybir.ActivationFunctionType

IN_NODES, OUT_NODES, F_SIZE = 4096, 1024, 16
CORES = 8
I_LOC = IN_NODES // CORES          # 512 in-nodes per core
ROW = OUT_NODES * F_SIZE           # 16384 floats per in-node row
P = 128
NBLK = I_LOC // P                  # 4 i-blocks per core
QT = 4096                          # streamed quarter width (elems/partition)
NQT = ROW // QT                    # 4 quarters per block
CH = 2048                          # chunk/piece width (elems/partition)
NCH_Q = QT // CH                   # 2 chunks per quarter
NMM = CH // 512                    # 4 matmuls per piece
F32R_MM = True                     # fast-path fp32 matmuls (1 cyc/row)
MM_DT = mybir.dt.float32r if F32R_MM else F32


def _body(nc, tc, uh, v_out, R, rg):
    uh_t = uh.rearrange("(n p) r -> n p r", p=P)   # [NBLK, 128, 16384]

    with ExitStack() as ctx:
        io = ctx.enter_context(tc.tile_pool(name="io", bufs=4))
        work = ctx.enter_context(tc.tile_pool(name="work", bufs=4))
        small = ctx.enter_context(tc.tile_pool(name="small", bufs=2))
        persist = ctx.enter_context(tc.tile_pool(name="persist", bufs=1))
        pspool = ctx.enter_context(tc.tile_pool(name="pspool", bufs=2, space="PSUM"))
        dram = ctx.enter_context(tc.tile_pool(name="dram", bufs=2, space="DRAM"))

        c0_f = persist.tile([P, 1], F32, name="c0_f")
        nc.vector.memset(c0_f, 1.0 / OUT_NODES)
        c0 = persist.tile([P, 1], MM_DT, name="c0")
        nc.vector.tensor_copy(c0, c0_f)
        w_sb = w_acc = None
        if R > 1:
            w_sb = persist.tile([P, ROW], F32, name="w_sb")
            w_acc = persist.tile([P, P], F32, name="w_acc")

        for t in range(1, R + 1):
            ar_in = dram.tile([NBLK, ROW], F32, tag="ar_in")
            for blk in range(NBLK):
                uts = []
                for q in range(NQT):
                    ut = io.tile([P, QT], F32, tag="ut")
                    nc.sync.dma_start(ut, uh_t[blk, :, q * QT:(q + 1) * QT])
                    uts.append(ut)
                if t == 1:
                    rinv = c0
                else:
                    b = small.tile([P, OUT_NODES], F32, tag="b")
                    for q in range(NQT):
                        for k in range(NCH_Q):
                            sl = slice(k * CH, (k + 1) * CH)
                            g0 = q * QT + k * CH
                            tm = work.tile([P, CH], F32, tag="tm")
                            # b-mul on GpSimd (concurrent with DVE TT/reduce)
                            nc.gpsimd.tensor_mul(
                                tm, uts[q][:, sl], w_sb[:, g0:g0 + CH])
                            o0 = g0 // F_SIZE
                            nc.vector.reduce_sum(
                                b[:, o0:o0 + CH // F_SIZE],
                                tm.rearrange("p (o f) -> p o f", f=F_SIZE),
                                axis=AX.X,
                            )
                    e = small.tile([P, OUT_NODES], F32, tag="e")
                    den = small.tile([P, 1], F32, tag="den")
                    nc.scalar.activation(e, b, AF.Exp, accum_out=den)
                    rinv_f = small.tile([P, 1], F32, tag="rinv_f")
                    nc.vector.reciprocal(rinv_f, den)
                    rinv = small.tile([P, 1], MM_DT, tag="rinv")
                    nc.vector.tensor_copy(rinv, rinv_f)
                for q in range(NQT):
                    for k in range(NCH_Q):
                        sl = slice(k * CH, (k + 1) * CH)
                        g0 = q * QT + k * CH
                        pt = work.tile([P, CH], MM_DT, tag="tm")
                        if t == 1:
                            # round to f32r on idle DVE (pass 1 only)
                            nc.vector.tensor_copy(pt, uts[q][:, sl])
                        else:
                            o0 = g0 // F_SIZE
                            och = CH // F_SIZE
                            nc.vector.tensor_mul(
                                pt.rearrange("p (o f) -> p o f", f=F_SIZE),
                                uts[q][:, sl].rearrange(
                                    "p (o f) -> p o f", f=F_SIZE),
                                e[:, o0:o0 + och][:, :, None].broadcast_to(
                                    [P, och, F_SIZE]),
                            )
                        ps = pspool.tile([1, CH], F32, tag="ps")
                        for c in range(NMM):
                            nc.tensor.matmul(
                                ps[:, c * 512:(c + 1) * 512],
                                rinv,
                                pt[:, c * 512:(c + 1) * 512],
                                start=True, stop=True,
                                skip_group_check=True,
                            )
                        fl = small.tile([1, CH], F32, tag="fl")
                        nc.scalar.copy(fl, ps)
                        nc.sync.dma_start(ar_in[blk, g0:g0 + CH], fl)
            ar_out = dram.tile([NBLK, ROW], F32, tag="ar_out")
            nc.gpsimd.collective_compute(
                "AllReduce", mybir.AluOpType.add, replica_groups=rg,
                ins=[ar_in.opt()], outs=[ar_out.opt()],
            )
            # s2[p,(j,f)] with o = p*8+j: sum the 4 block rows post-AR
            slds = []
            for blk in range(NBLK):
                sld = small.tile([P, P], F32, tag="sld", bufs=4)
                nc.sync.dma_start(
                    sld, ar_out[blk].rearrange("(p q) -> p q", p=P))
                slds.append(sld)
            s2 = small.tile([P, P], F32, tag="s2")
            nc.vector.tensor_add(s2, slds[0], slds[1])
            nc.vector.tensor_add(s2, s2, slds[2])
            nc.vector.tensor_add(s2, s2, slds[3])
            # squash: v = s * sqrt(sq)/(1+sq), sq = sum_f s^2
            ssq = small.tile([P, P], F32, tag="ssq")
            nc.vector.tensor_mul(ssq, s2, s2)
            sq = small.tile([P, 8], F32, tag="sq")
            nc.vector.reduce_sum(
                sq, ssq.rearrange("p (j f) -> p j f", f=F_SIZE), axis=AX.X)
            # sqrt via exp(0.5*ln(x)): stays in the exp/ln ACT table set
            lnq = small.tile([P, 8], F32, tag="lnq")
            nc.scalar.activation(lnq, sq, AF.Ln)
            y = small.tile([P, 8], F32, tag="y")
            nc.scalar.activation(y, lnq, AF.Exp, scale=0.5)
            # one Newton step: y <- 0.5*(y + sq/y)
            ry = small.tile([P, 8], F32, tag="ry")
            nc.vector.reciprocal(ry, y)
            t1 = small.tile([P, 8], F32, tag="t1")
            nc.vector.tensor_mul(t1, sq, ry)
            nc.vector.tensor_add(t1, t1, y)
            nc.vector.tensor_scalar_mul(t1, t1, 0.5)
            d1 = small.tile([P, 8], F32, tag="d1")
            nc.vector.tensor_scalar_add(d1, sq, 1.0)
            rd = small.tile([P, 8], F32, tag="rd")
            nc.vector.reciprocal(rd, d1)
            sc = small.tile([P, 8], F32, tag="sc")
            nc.vector.tensor_mul(sc, t1, rd)
            v_sb = small.tile([P, P], F32, tag="v_sb")
            nc.vector.tensor_mul(
                v_sb.rearrange("p (j f) -> p j f", f=F_SIZE),
                s2.rearrange("p (j f) -> p j f", f=F_SIZE),
                sc[:, :, None].broadcast_to([P, 8, F_SIZE]),
            )
            if t == R:
                nc.sync.dma_start(
                    v_out.rearrange("(p j) f -> p (j f)", j=8), v_sb)
            else:
                if t == 1:
                    nc.scalar.copy(w_acc, v_sb)
                else:
                    nc.vector.tensor_add(w_acc, w_acc, v_sb)
                # broadcast w to all partitions via DRAM round-trip:
                # w_acc[p,(j,f)] -> flat w_dram[o*16+f] -> [128, ROW] bcast
                w_dram = dram.tile([ROW], F32, tag="w_dram")
                nc.sync.dma_start(
                    w_dram.rearrange("(p q) -> p q", p=P), w_acc)
                wd_b = w_dram.unsqueeze(0)
                for j in range(8):
                    sl = slice(j * CH, (j + 1) * CH)
                    nc.sync.dma_start(
                        w_sb[:, sl],
                        wd_b[:, sl].broadcast_to([P, CH]))


def _build(routing_num: int):
    R = int(routing_num)
    assert R >= 1
    nc = bacc.Bacc(
        "TRN2", target_bir_lowering=False, debug=False, num_devices=CORES)
    uh = nc.dram_tensor("uh", [I_LOC, ROW], F32, kind="ExternalInput")
    v_out = nc.dram_tensor("v_out", [OUT_NODES, F_SIZE], F32,
                           kind="ExternalOutput")
    rg = [list(range(CORES))]
    with tile.TileContext(nc) as tc:
        _body(nc, tc, uh.ap(), v_out.ap(), R, rg)
    nc.compile()
    return nc


_CACHE: dict = {}


def _get_nc(routing_num: int):
    R = int(routing_num)
    if R not in _CACHE:
        _CACHE[R] = _build(R)
    return _CACHE[R]


def _shard(u_hat: np.ndarray):
    uh = np.ascontiguousarray(np.asarray(u_hat, dtype=np.float32))
    assert uh.shape == (IN_NODES * OUT_NODES, F_SIZE), uh.shape
    uh = uh.reshape(IN_NODES, ROW)
    return [
        {"uh": np.ascontiguousarray(uh[k * I_LOC:(k + 1) * I_LOC])}
        for k in range(CORES)
    ]


def run(u_hat, routing_num, trace=False):
    nc = _get_nc(routing_num)
    in_maps = _shard(u_hat)
    res = bass_utils.run_bass_kernel_spmd(
        nc, in_maps, core_ids=list(range(CORES)), trace=trace)
    return res


def kernel(u_hat, routing_num):
    res = run(u_hat, routing_num, trace=False)
    return np.asarray(res.results[0]["v_out"], dtype=np.float32)



# revision 8
# speedup vs baseline: 1.2149x; 1.2149x over previous
"""DGL capsule routing layer on 8 trn2 NeuronCores (Bass/Tile).

Math: for routing_num iterations,
    c = softmax(b, axis=out)                        # b0 = 0
    s = einsum('io,iof->of', c, uh)
    v = squash(s)
    b = b + einsum('iof,of->io', uh, v)
Output: final v [OUT, F].

Key identity: b_t = uh . (v_1 + ... + v_{t-1}) (b is linear in uh), so b is
never materialized across iterations; each iteration is one pass over uh with
w_t = cumulative sum of v's:
    pass t: b = sum_f uh[i,o,f]*w[o,f]; e = exp(b); r_i = 1/sum_o e
            s[o,f] = sum_i r_i * e[i,o] * uh[i,o,f]   (partial per core)
            AllReduce(s); v = squash(s); w += v
Pass 1 has c uniform (=1/OUT) so it is a pure PE pass.

This version vs the f32 streaming baseline:
  - u_hat is cast to bf16 on the host; the 16 MiB/core shard is DMA'd to
    SBUF once in pass 1 and stays resident, so passes 2..R do no u_hat HBM
    traffic at all (baseline re-streamed 32 MiB f32 every pass).
  - all big elementwise ops (uh*w on GpSimd, e*uh on DVE) and the f-segmented
    reduce run on bf16 operands: 2x DVE/GpSimd throughput vs f32.
  - s-partials for block pairs accumulate in one PSUM tile (matmul accum
    across the two stationaries), halving the 1-lane PSUM->SBUF flushes and
    the AllReduce payload; the AR itself runs in bf16 ([2,16384] = 64 KiB).

Sharding: i (in_nodes) split across 8 cores, 512 rows each (4 blocks of
128 partitions). Engine plan per pass >= 2:
  GpSimd: tm = uh_bf * w_bf           (bf16 2-input mul)
  DVE:    b-slice = segsum_f(tm);  p = e * uh_bf (e bcast over f)
  ACT:    e = exp(b) with fused denominator accum; psum flushes
  PE:     s-partial += rinv[i]*p[i,:] as 512-col bf16 matmuls, two i-blocks
          accumulated per [1,2048] psum tile -> DRAM [2,16384] -> AllReduce.
"""

import numpy as np
import ml_dtypes
from contextlib import ExitStack

import concourse.bass as bass
import concourse.mybir as mybir
import concourse.tile as tile
from concourse import bacc
from concourse import bass_utils

F32 = mybir.dt.float32
BF16 = mybir.dt.bfloat16
AX = mybir.AxisListType
AF = mybir.ActivationFunctionType

IN_NODES, OUT_NODES, F_SIZE = 4096, 1024, 16
CORES = 8
I_LOC = IN_NODES // CORES          # 512 in-nodes per core
ROW = OUT_NODES * F_SIZE           # 16384 elems per in-node row
P = 128
NBLK = I_LOC // P                  # 4 i-blocks per core
NPAIR = NBLK // 2                  # 2 block pairs
CH = 2048                          # elementwise chunk width
NCH = ROW // CH                    # 8 chunks per block row
SEG = 2048                         # psum segment width
NSEG = ROW // SEG                  # 8 segments per block row
MMW = 512                          # matmul moving width


def _body(nc, tc, uh, v_out, R, rg):
    uh_t = uh.rearrange("(n p) r -> n p r", p=P)   # [NBLK, 128, 16384] bf16

    with ExitStack() as ctx:
        persist = ctx.enter_context(tc.tile_pool(name="persist", bufs=1))
        tmpool = ctx.enter_context(tc.tile_pool(name="tmpool", bufs=2))
        ppool = ctx.enter_context(tc.tile_pool(name="ppool", bufs=2))
        small = ctx.enter_context(tc.tile_pool(name="small", bufs=2))
        pspool = ctx.enter_context(tc.tile_pool(name="pspool", bufs=2, space="PSUM"))
        dram = ctx.enter_context(tc.tile_pool(name="dram", bufs=2, space="DRAM"))

        c0 = persist.tile([P, 1], BF16, name="c0")
        nc.vector.memset(c0, 1.0 / OUT_NODES)
        uh_sb = [persist.tile([P, ROW], BF16, name=f"uh{b}") for b in range(NBLK)]
        w_sb = w_acc = None
        if R > 1:
            w_sb = persist.tile([P, ROW], BF16, name="w_sb")
            w_acc = persist.tile([P, P], F32, name="w_acc")

        # resident uh upload (pass 1 overlaps matmuls with these loads)
        for blk in range(NBLK):
            nc.sync.dma_start(uh_sb[blk], uh_t[blk])

        w_dram = None
        for t in range(1, R + 1):
            ar_in = dram.tile([NPAIR, ROW], BF16, tag="ar_in")
            if t > 1:
                # broadcast w (written to w_dram at the end of pass t-1)
                # to all 128 partitions, chunk by chunk
                wd_b = w_dram.unsqueeze(0)
                for c in range(NCH):
                    sl = slice(c * CH, (c + 1) * CH)
                    nc.sync.dma_start(
                        w_sb[:, sl], wd_b[:, sl].broadcast_to([P, CH]))
            for pair in range(NPAIR):
                es, rinvs = [None, None], [None, None]
                if t > 1:
                    # phase A: b = segsum_f(uh*w); e = exp(b); rinv = 1/sum e
                    for j in range(2):
                        blk = 2 * pair + j
                        b_t = small.tile([P, OUT_NODES], F32, tag="b")
                        for c in range(NCH):
                            sl = slice(c * CH, (c + 1) * CH)
                            tm = tmpool.tile([P, CH], BF16, tag="tm")
                            nc.gpsimd.tensor_mul(tm, uh_sb[blk][:, sl], w_sb[:, sl])
                            nc.vector.reduce_sum(
                                b_t[:, c * (CH // F_SIZE):(c + 1) * (CH // F_SIZE)],
                                tm.rearrange("p (o f) -> p o f", f=F_SIZE),
                                axis=AX.X,
                            )
                        e_t = small.tile([P, OUT_NODES], BF16, tag="e", bufs=4)
                        den = small.tile([P, 1], F32, tag="den", bufs=4)
                        nc.scalar.activation(e_t, b_t, AF.Exp, accum_out=den)
                        rinv_f = small.tile([P, 1], F32, tag="rinv_f", bufs=4)
                        nc.vector.reciprocal(rinv_f, den)
                        rinv_b = small.tile([P, 1], BF16, tag="rinv", bufs=4)
                        nc.vector.tensor_copy(rinv_b, rinv_f)
                        es[j], rinvs[j] = e_t, rinv_b
                # phase B: p = e*uh; psum[1,SEG] += rinv . p  (both blocks)
                for s in range(NSEG):
                    ss = slice(s * SEG, (s + 1) * SEG)
                    ps = pspool.tile([1, SEG], F32, tag="ps")
                    for j in range(2):
                        blk = 2 * pair + j
                        if t == 1:
                            mv, stat = uh_sb[blk], c0
                        else:
                            pp = ppool.tile([P, SEG], BF16, tag="pp")
                            och = SEG // F_SIZE
                            nc.vector.tensor_mul(
                                pp.rearrange("p (o f) -> p o f", f=F_SIZE),
                                uh_sb[blk][:, ss].rearrange(
                                    "p (o f) -> p o f", f=F_SIZE),
                                es[j][:, s * och:(s + 1) * och][:, :, None]
                                .broadcast_to([P, och, F_SIZE]),
                            )
                            mv, stat = None, rinvs[j]
                        for cix in range(SEG // MMW):
                            msl = slice(cix * MMW, (cix + 1) * MMW)
                            src = (mv[:, s * SEG + cix * MMW:
                                      s * SEG + (cix + 1) * MMW]
                                   if t == 1 else pp[:, msl])
                            nc.tensor.matmul(
                                ps[:, msl], stat, src,
                                start=(j == 0), stop=(j == 1),
                                skip_group_check=True,
                            )
                    fl = small.tile([1, SEG], BF16, tag="fl")
                    nc.scalar.copy(fl, ps)
                    nc.sync.dma_start(ar_in[pair, ss], fl)
            ar_out = dram.tile([NPAIR, ROW], BF16, tag="ar_out")
            nc.gpsimd.collective_compute(
                "AllReduce", mybir.AluOpType.add, replica_groups=rg,
                ins=[ar_in.opt()], outs=[ar_out.opt()],
            )
            # s2[p,(j,f)] with o = p*8+j: sum the pair rows post-AR
            slds = []
            for r in range(NPAIR):
                sld = small.tile([P, P], BF16, tag="sld")
                nc.sync.dma_start(
                    sld, ar_out[r].rearrange("(p q) -> p q", p=P))
                slds.append(sld)
            s2 = small.tile([P, P], F32, tag="s2")
            nc.vector.tensor_add(s2, slds[0], slds[1])
            # squash: v = s * sqrt(sq)/(1+sq), sq = sum_f s^2
            ssq = small.tile([P, P], F32, tag="ssq")
            nc.vector.tensor_mul(ssq, s2, s2)
            sq = small.tile([P, 8], F32, tag="sq")
            nc.vector.reduce_sum(
                sq, ssq.rearrange("p (j f) -> p j f", f=F_SIZE), axis=AX.X)
            # sqrt via exp(0.5*ln(x)): stays in the exp/ln ACT table set
            lnq = small.tile([P, 8], F32, tag="lnq")
            nc.scalar.activation(lnq, sq, AF.Ln)
            y = small.tile([P, 8], F32, tag="y")
            nc.scalar.activation(y, lnq, AF.Exp, scale=0.5)
            # one Newton step: y <- 0.5*(y + sq/y)
            ry = small.tile([P, 8], F32, tag="ry")
            nc.vector.reciprocal(ry, y)
            t1 = small.tile([P, 8], F32, tag="t1")
            nc.vector.tensor_mul(t1, sq, ry)
            nc.vector.tensor_add(t1, t1, y)
            nc.vector.tensor_scalar_mul(t1, t1, 0.5)
            d1 = small.tile([P, 8], F32, tag="d1")
            nc.vector.tensor_scalar_add(d1, sq, 1.0)
            rd = small.tile([P, 8], F32, tag="rd")
            nc.vector.reciprocal(rd, d1)
            sc = small.tile([P, 8], F32, tag="sc")
            nc.vector.tensor_mul(sc, t1, rd)
            v_sb = small.tile([P, P], F32, tag="v_sb")
            nc.vector.tensor_mul(
                v_sb.rearrange("p (j f) -> p j f", f=F_SIZE),
                s2.rearrange("p (j f) -> p j f", f=F_SIZE),
                sc[:, :, None].broadcast_to([P, 8, F_SIZE]),
            )
            if t == R:
                nc.sync.dma_start(
                    v_out.rearrange("(p j) f -> p (j f)", j=8), v_sb)
            else:
                if t == 1:
                    nc.scalar.copy(w_acc, v_sb)
                else:
                    nc.vector.tensor_add(w_acc, w_acc, v_sb)
                # w_acc[p,(j,f)] -> flat w_dram[o*16+f] (bf16), broadcast
                # back per-chunk at the start of the next pass's phase A
                w_bf = small.tile([P, P], BF16, tag="w_bf")
                nc.vector.tensor_copy(w_bf, w_acc)
                w_dram = dram.tile([ROW], BF16, tag="w_dram")
                nc.sync.dma_start(
                    w_dram.rearrange("(p q) -> p q", p=P), w_bf)


def _build(routing_num: int):
    R = int(routing_num)
    assert R >= 1
    nc = bacc.Bacc(
        "TRN2", target_bir_lowering=False, debug=False, num_devices=CORES)
    uh = nc.dram_tensor("uh", [I_LOC, ROW], BF16, kind="ExternalInput")
    v_out = nc.dram_tensor("v_out", [OUT_NODES, F_SIZE], F32,
                           kind="ExternalOutput")
    rg = [list(range(CORES))]
    with tile.TileContext(nc) as tc:
        _body(nc, tc, uh.ap(), v_out.ap(), R, rg)
    nc.compile()
    return nc


_CACHE: dict = {}


def _get_nc(routing_num: int):
    R = int(routing_num)
    if R not in _CACHE:
        _CACHE[R] = _build(R)
    return _CACHE[R]


def _shard(u_hat: np.ndarray):
    uh = np.asarray(u_hat, dtype=np.float32)
    assert uh.shape == (IN_NODES * OUT_NODES, F_SIZE), uh.shape
    uh = uh.reshape(IN_NODES, ROW).astype(ml_dtypes.bfloat16)
    return [
        {"uh": np.ascontiguousarray(uh[k * I_LOC:(k + 1) * I_LOC])}
        for k in range(CORES)
    ]


def run(u_hat, routing_num, trace=False):
    nc = _get_nc(routing_num)
    in_maps = _shard(u_hat)
    res = bass_utils.run_bass_kernel_spmd(
        nc, in_maps, core_ids=list(range(CORES)), trace=trace)
    return res


def kernel(u_hat, routing_num):
    res = run(u_hat, routing_num, trace=False)
    return np.asarray(res.results[0]["v_out"], dtype=np.float32)


# revision 9
# speedup vs baseline: 1.2712x; 1.0464x over previous
"""DGL capsule routing layer on 8 trn2 NeuronCores (Bass/Tile).

Math: for routing_num iterations,
    c = softmax(b, axis=out)                        # b0 = 0
    s = einsum('io,iof->of', c, uh)
    v = squash(s)
    b = b + einsum('iof,of->io', uh, v)
Output: final v [OUT, F].

Key identity: b_t = uh . (v_1 + ... + v_{t-1}) (b is linear in uh), so b is
never materialized across iterations; each iteration is one pass over uh with
w_t = cumulative sum of v's:
    pass t: b = sum_f uh[i,o,f]*w[o,f]; e = exp(b); r_i = 1/sum_o e
            s[o,f] = sum_i r_i * e[i,o] * uh[i,o,f]   (partial per core)
            AllReduce(s); v = squash(s); w += v
Pass 1 has c uniform (=1/OUT) so it is a pure PE pass.

This version vs the f32 streaming baseline:
  - u_hat is cast to bf16 on the host; the 16 MiB/core shard is DMA'd to
    SBUF once in pass 1 and stays resident, so passes 2..R do no u_hat HBM
    traffic at all (baseline re-streamed 32 MiB f32 every pass).
  - all big elementwise ops (uh*w on GpSimd, e*uh on DVE) and the f-segmented
    reduce run on bf16 operands: 2x DVE/GpSimd throughput vs f32.
  - s-partials for block pairs accumulate in one PSUM tile (matmul accum
    across the two stationaries), halving the 1-lane PSUM->SBUF flushes and
    the AllReduce payload; the AR itself runs in bf16 ([2,16384] = 64 KiB).

Sharding: i (in_nodes) split across 8 cores, 512 rows each (4 blocks of
128 partitions). Engine plan per pass >= 2:
  GpSimd: tm = uh_bf * w_bf           (bf16 2-input mul)
  DVE:    b-slice = segsum_f(tm);  p = e * uh_bf (e bcast over f)
  ACT:    e = exp(b) with fused denominator accum; psum flushes
  PE:     s-partial += rinv[i]*p[i,:] as 512-col bf16 matmuls, two i-blocks
          accumulated per [1,2048] psum tile -> DRAM [2,16384] -> AllReduce.
"""

import numpy as np
import ml_dtypes
from contextlib import ExitStack

import concourse.bass as bass
import concourse.mybir as mybir
import concourse.tile as tile
from concourse import bacc
from concourse import bass_utils

F32 = mybir.dt.float32
BF16 = mybir.dt.bfloat16
AX = mybir.AxisListType
AF = mybir.ActivationFunctionType

IN_NODES, OUT_NODES, F_SIZE = 4096, 1024, 16
CORES = 8
I_LOC = IN_NODES // CORES          # 512 in-nodes per core
ROW = OUT_NODES * F_SIZE           # 16384 elems per in-node row
P = 128
NBLK = I_LOC // P                  # 4 i-blocks per core
NPAIR = NBLK // 2                  # 2 block pairs
CH = 2048                          # elementwise chunk width
NCH = ROW // CH                    # 8 chunks per block row
SEG = 2048                         # psum segment width
NSEG = ROW // SEG                  # 8 segments per block row
MMW = 512                          # matmul moving width


def _body(nc, tc, uh, v_out, R, rg):
    uh_t = uh.rearrange("(n p) r -> n p r", p=P)   # [NBLK, 128, 16384] bf16

    with ExitStack() as ctx:
        persist = ctx.enter_context(tc.tile_pool(name="persist", bufs=1))
        tmpool = ctx.enter_context(tc.tile_pool(name="tmpool", bufs=2))
        ppool = ctx.enter_context(tc.tile_pool(name="ppool", bufs=2))
        small = ctx.enter_context(tc.tile_pool(name="small", bufs=2))
        pspool = ctx.enter_context(tc.tile_pool(name="pspool", bufs=2, space="PSUM"))
        dram = ctx.enter_context(tc.tile_pool(name="dram", bufs=2, space="DRAM"))

        c0 = persist.tile([P, 1], BF16, name="c0")
        nc.vector.memset(c0, 1.0 / OUT_NODES)
        uh_sb = [persist.tile([P, ROW], BF16, name=f"uh{b}") for b in range(NBLK)]
        w_sb = w_acc = None
        if R > 1:
            w_sb = persist.tile([P, ROW], BF16, name="w_sb")
            w_acc = persist.tile([P, P], F32, name="w_acc")

        # resident uh upload (pass 1 overlaps matmuls with these loads)
        for blk in range(NBLK):
            nc.sync.dma_start(uh_sb[blk], uh_t[blk])

        w_dram = None
        for t in range(1, R + 1):
            ar_in = dram.tile([NPAIR, ROW], BF16, tag="ar_in")
            if t > 1:
                # broadcast w (written to w_dram at the end of pass t-1)
                # to all 128 partitions, chunk by chunk
                wd_b = w_dram.unsqueeze(0)
                for c in range(NCH):
                    sl = slice(c * CH, (c + 1) * CH)
                    nc.sync.dma_start(
                        w_sb[:, sl], wd_b[:, sl].broadcast_to([P, CH]))
            for pair in range(NPAIR):
                es, rinvs = [None, None], [None, None]
                if t > 1:
                    # phase A: b = segsum_f(uh*w); e = exp(b); rinv = 1/sum e
                    for j in range(2):
                        blk = 2 * pair + j
                        b_t = small.tile([P, OUT_NODES], F32, tag="b")
                        for c in range(NCH):
                            sl = slice(c * CH, (c + 1) * CH)
                            tm = tmpool.tile([P, CH], BF16, tag="tm")
                            eng = nc.gpsimd if c % 2 == 0 else nc.vector
                            eng.tensor_mul(tm, uh_sb[blk][:, sl], w_sb[:, sl])
                            nc.vector.reduce_sum(
                                b_t[:, c * (CH // F_SIZE):(c + 1) * (CH // F_SIZE)],
                                tm.rearrange("p (o f) -> p o f", f=F_SIZE),
                                axis=AX.X,
                            )
                        e_t = small.tile([P, OUT_NODES], BF16, tag="e", bufs=4)
                        den = small.tile([P, 1], F32, tag="den", bufs=4)
                        nc.scalar.activation(e_t, b_t, AF.Exp, accum_out=den)
                        rinv_f = small.tile([P, 1], F32, tag="rinv_f", bufs=4)
                        nc.vector.reciprocal(rinv_f, den)
                        rinv_b = small.tile([P, 1], BF16, tag="rinv", bufs=4)
                        nc.vector.tensor_copy(rinv_b, rinv_f)
                        es[j], rinvs[j] = e_t, rinv_b
                # phase B: p = e*uh; psum[1,SEG] += rinv . p  (both blocks)
                for s in range(NSEG):
                    ss = slice(s * SEG, (s + 1) * SEG)
                    ps = pspool.tile([1, SEG], F32, tag="ps")
                    for j in range(2):
                        blk = 2 * pair + j
                        if t == 1:
                            mv, stat = uh_sb[blk], c0
                        else:
                            pp = ppool.tile([P, SEG], BF16, tag="pp")
                            och = SEG // F_SIZE
                            nc.vector.tensor_mul(
                                pp.rearrange("p (o f) -> p o f", f=F_SIZE),
                                uh_sb[blk][:, ss].rearrange(
                                    "p (o f) -> p o f", f=F_SIZE),
                                es[j][:, s * och:(s + 1) * och][:, :, None]
                                .broadcast_to([P, och, F_SIZE]),
                            )
                            mv, stat = None, rinvs[j]
                        for cix in range(SEG // MMW):
                            msl = slice(cix * MMW, (cix + 1) * MMW)
                            src = (mv[:, s * SEG + cix * MMW:
                                      s * SEG + (cix + 1) * MMW]
                                   if t == 1 else pp[:, msl])
                            nc.tensor.matmul(
                                ps[:, msl], stat, src,
                                start=(j == 0), stop=(j == 1),
                                skip_group_check=True,
                            )
                    fl = small.tile([1, SEG], BF16, tag="fl")
                    nc.scalar.copy(fl, ps)
                    nc.sync.dma_start(ar_in[pair, ss], fl)
            ar_out = dram.tile([NPAIR, ROW], BF16, tag="ar_out")
            nc.gpsimd.collective_compute(
                "AllReduce", mybir.AluOpType.add, replica_groups=rg,
                ins=[ar_in.opt()], outs=[ar_out.opt()],
            )
            # s2[p,(j,f)] with o = p*8+j: sum the pair rows post-AR
            slds = []
            for r in range(NPAIR):
                sld = small.tile([P, P], BF16, tag="sld")
                nc.sync.dma_start(
                    sld, ar_out[r].rearrange("(p q) -> p q", p=P))
                slds.append(sld)
            s2 = small.tile([P, P], F32, tag="s2")
            nc.vector.tensor_add(s2, slds[0], slds[1])
            # squash: v = s * sqrt(sq)/(1+sq), sq = sum_f s^2
            ssq = small.tile([P, P], F32, tag="ssq")
            nc.vector.tensor_mul(ssq, s2, s2)
            sq = small.tile([P, 8], F32, tag="sq")
            nc.vector.reduce_sum(
                sq, ssq.rearrange("p (j f) -> p j f", f=F_SIZE), axis=AX.X)
            # sqrt via exp(0.5*ln(x)): stays in the exp/ln ACT table set
            lnq = small.tile([P, 8], F32, tag="lnq")
            nc.scalar.activation(lnq, sq, AF.Ln)
            y = small.tile([P, 8], F32, tag="y")
            nc.scalar.activation(y, lnq, AF.Exp, scale=0.5)
            # one Newton step: y <- 0.5*(y + sq/y)
            ry = small.tile([P, 8], F32, tag="ry")
            nc.vector.reciprocal(ry, y)
            t1 = small.tile([P, 8], F32, tag="t1")
            nc.vector.tensor_mul(t1, sq, ry)
            nc.vector.tensor_add(t1, t1, y)
            nc.vector.tensor_scalar_mul(t1, t1, 0.5)
            d1 = small.tile([P, 8], F32, tag="d1")
            nc.vector.tensor_scalar_add(d1, sq, 1.0)
            rd = small.tile([P, 8], F32, tag="rd")
            nc.vector.reciprocal(rd, d1)
            sc = small.tile([P, 8], F32, tag="sc")
            nc.vector.tensor_mul(sc, t1, rd)
            v_sb = small.tile([P, P], F32, tag="v_sb")
            nc.vector.tensor_mul(
                v_sb.rearrange("p (j f) -> p j f", f=F_SIZE),
                s2.rearrange("p (j f) -> p j f", f=F_SIZE),
                sc[:, :, None].broadcast_to([P, 8, F_SIZE]),
            )
            if t == R:
                nc.sync.dma_start(
                    v_out.rearrange("(p j) f -> p (j f)", j=8), v_sb)
            else:
                if t == 1:
                    nc.scalar.copy(w_acc, v_sb)
                else:
                    nc.vector.tensor_add(w_acc, w_acc, v_sb)
                # w_acc[p,(j,f)] -> flat w_dram[o*16+f] (bf16), broadcast
                # back per-chunk at the start of the next pass's phase A
                w_bf = small.tile([P, P], BF16, tag="w_bf")
                nc.vector.tensor_copy(w_bf, w_acc)
                w_dram = dram.tile([ROW], BF16, tag="w_dram")
                nc.sync.dma_start(
                    w_dram.rearrange("(p q) -> p q", p=P), w_bf)


def _build(routing_num: int):
    R = int(routing_num)
    assert R >= 1
    nc = bacc.Bacc(
        "TRN2", target_bir_lowering=False, debug=False, num_devices=CORES)
    uh = nc.dram_tensor("uh", [I_LOC, ROW], BF16, kind="ExternalInput")
    v_out = nc.dram_tensor("v_out", [OUT_NODES, F_SIZE], F32,
                           kind="ExternalOutput")
    rg = [list(range(CORES))]
    with tile.TileContext(nc) as tc:
        _body(nc, tc, uh.ap(), v_out.ap(), R, rg)
    nc.compile()
    return nc


_CACHE: dict = {}


def _get_nc(routing_num: int):
    R = int(routing_num)
    if R not in _CACHE:
        _CACHE[R] = _build(R)
    return _CACHE[R]


def _shard(u_hat: np.ndarray):
    uh = np.asarray(u_hat, dtype=np.float32)
    assert uh.shape == (IN_NODES * OUT_NODES, F_SIZE), uh.shape
    uh = uh.reshape(IN_NODES, ROW).astype(ml_dtypes.bfloat16)
    return [
        {"uh": np.ascontiguousarray(uh[k * I_LOC:(k + 1) * I_LOC])}
        for k in range(CORES)
    ]


def run(u_hat, routing_num, trace=False):
    nc = _get_nc(routing_num)
    in_maps = _shard(u_hat)
    res = bass_utils.run_bass_kernel_spmd(
        nc, in_maps, core_ids=list(range(CORES)), trace=trace)
    return res


def kernel(u_hat, routing_num):
    res = run(u_hat, routing_num, trace=False)
    return np.asarray(res.results[0]["v_out"], dtype=np.float32)
